# revision 7
# baseline (speedup 1.0000x reference)
"""Trainium2 Bass kernel for additive (Bahdanau) attention.

Reference computation (B=32, L=2, S=4096, H=1024):
    q   = query[:, -1, :]                     # [B, H]
    wq  = tanh(q @ W^T)                       # [B, H]
    uk  = keys @ U^T                          # [B, S, H]
    scores[b, s] = V . (wq[b] + uk[b, s])     # [B, S]
    weight = softmax(scores, axis=-1)         # [B, S]
    context = weight @ keys                   # [B, H]
    returns (context[B,1,H], weight[B,1,S])

Algebraic optimization used here:
    scores[b, s] = (V . wq[b]) + (V @ U) . keys[b, s]
The first term is constant in s, and softmax is shift-invariant, so it
cancels exactly: weight = softmax(vU . keys[b, s]) with vU = V @ U.
Hence W and query do not affect the output at all (dead code), and the
275-GFLOP keys @ U^T einsum collapses to a matrix-vector product.
The kernel is then HBM-bound: keys are read exactly once.

Sharding: data-parallel over batch B across 8 cores (4 batches/core).
U, V are replicated; each core computes vU = V @ U on-device (tiny).

Per-core device program:
  setup: DMA U (8x[128,1024]) + V^T chunks; vU = V@U via 16 PE matmuls
         (lhsT = V^T chunk [128,1], rhs = U tile [128,512]); broadcast
         vU to all 128 partitions (GPSIMD partition_broadcast).
  per batch b (4x):
    - 32 keys tiles [128 s, 1024 h] streamed into a 40-slot SBUF ring
    - scores: fused DVE tensor_tensor_reduce (keys * vU, reduce over h)
      -> scores[128, 32]  (s = t*128 + p)
    - softmax: DVE free-dim max, GPSIMD partition all-reduce (max),
      ACT exp with fused row-sum accum, GPSIMD partition all-reduce
      (add), DVE reciprocal, DVE tensor_scalar_mul
    - weight out: DVE 32x32 block transpose + strided DMA
    - context: 64 PE matmuls (lhsT = weight column [128,1], rhs = keys
      tile halves [128,512]) accumulated in PSUM; ACT copy to SBUF; DMA
"""

import numpy as np

import concourse.bass as bass
import concourse.tile as tile
from concourse import bacc, bass_isa, mybir
from concourse.bass_utils import run_bass_kernel_spmd

# Problem shape (hardcoded; kernel.py must be self-contained)
B, L, S, H = 32, 2, 4096, 1024
N_CORES = 8
B_LOC = B // N_CORES          # 4 batches per core
P = 128                       # SBUF partitions
N_TILES = S // P              # 32 s-tiles per batch
KEY_BUFS = 40                 # keys ring slots (40 * 4KB/part = 160KB/part)

FP32 = mybir.dt.float32


def _emit(tc: tile.TileContext, repeat: int = 1):
    nc = tc.nc
    keys = nc.dram_tensor("keys", (B_LOC, S, H), FP32, kind="ExternalInput").ap()
    U = nc.dram_tensor("U", (H, H), FP32, kind="ExternalInput").ap()
    V = nc.dram_tensor("V", (1, H), FP32, kind="ExternalInput").ap()
    ctx_out = nc.dram_tensor("ctx_out", (B_LOC, H), FP32, kind="ExternalOutput").ap()
    wt_out = nc.dram_tensor("wt_out", (B_LOC, S), FP32, kind="ExternalOutput").ap()

    KH = H // P  # 8 contraction chunks for vU

    with (
        tc.tile_pool(name="keysp", bufs=KEY_BUFS) as keysp,
        tc.tile_pool(name="consts", bufs=2) as consts,
        tc.tile_pool(name="scratch", bufs=2) as scratch,
        tc.tile_pool(name="smalls", bufs=12) as smalls,
        tc.tile_pool(name="vt", bufs=8) as vtp,
        tc.tile_pool(name="scoresp", bufs=4) as scoresp,
        tc.tile_pool(name="expp", bufs=2) as expp,
        tc.tile_pool(name="wp", bufs=2) as wp,
        tc.tile_pool(name="wblkp", bufs=2) as wblkp,
        tc.tile_pool(name="ctxsb", bufs=2) as ctxsbp,
        tc.tile_pool(name="psum_v", bufs=2, space="PSUM") as psum_v,
        tc.tile_pool(name="psum_ctx", bufs=4, space="PSUM") as psum_ctx,
    ):
      for _rep in range(repeat):
        # ---------------- setup: vU = V @ U ----------------
        # V^T chunks: [128, 1] slices of V (partition = o within chunk)
        v_chunks = V.rearrange("a (k p) -> k p a", p=P)  # [8, 128, 1]
        vu_lo = psum_v.tile([1, 512], FP32, tag="vul")
        vu_hi = psum_v.tile([1, 512], FP32, tag="vuh")
        for k in range(KH):
            vt = vtp.tile([P, 1], FP32, tag="vt")
            nc.sync.dma_start(out=vt, in_=v_chunks[k])
            ut = keysp.tile([P, H], FP32, tag="keys")
            nc.sync.dma_start(out=ut, in_=U[k * P : (k + 1) * P, :])
            nc.tensor.matmul(vu_lo, lhsT=vt, rhs=ut[:, 0:512],
                             start=(k == 0), stop=(k == KH - 1))
            nc.tensor.matmul(vu_hi, lhsT=vt, rhs=ut[:, 512:1024],
                             start=(k == 0), stop=(k == KH - 1))
        vu_row = consts.tile([1, H], FP32)
        nc.scalar.copy(vu_row[:, 0:512], vu_lo)
        nc.scalar.copy(vu_row[:, 512:1024], vu_hi)
        vu_bcast = consts.tile([P, H], FP32)
        nc.gpsimd.partition_broadcast(vu_bcast, vu_row)

        # ---------------- main loop over local batches ----------------
        for b in range(B_LOC):
            # scores pass: stream keys, fused multiply+reduce
            ktiles = []
            scores = scoresp.tile([P, N_TILES], FP32, tag="scores")
            for t in range(N_TILES):
                kt = keysp.tile([P, H], FP32, tag="keys")
                nc.sync.dma_start(out=kt, in_=keys[b, t * P : (t + 1) * P, :])
                ktiles.append(kt)
                # keys * vU on DVE; free-dim sum via ACT copy-with-accum
                # (tensor_tensor_reduce faults on this HW path)
                prod = scratch.tile([P, H], FP32, tag="prod")
                nc.vector.tensor_mul(prod, kt, vu_bcast)
                nc.scalar.activation(
                    out=prod, in_=prod,
                    func=mybir.ActivationFunctionType.Copy,
                    accum_out=scores[:, t : t + 1],
                )

            # softmax over 4096 = [128 partitions x 32 free]
            m1 = smalls.tile([P, 1], FP32, tag="m1")
            nc.vector.tensor_reduce(out=m1, in_=scores,
                                    axis=mybir.AxisListType.X,
                                    op=mybir.AluOpType.max)
            gmax = smalls.tile([P, 1], FP32, tag="gmax")
            nc.gpsimd.partition_all_reduce(gmax, m1, channels=P,
                                           reduce_op=bass_isa.ReduceOp.max)
            negmax = smalls.tile([P, 1], FP32, tag="negmax")
            nc.vector.tensor_scalar_mul(out=negmax, in0=gmax, scalar1=-1.0)
            exp_t = expp.tile([P, N_TILES], FP32, tag="exp")
            rowsum = smalls.tile([P, 1], FP32, tag="rowsum")
            nc.scalar.activation(out=exp_t, in_=scores,
                                 func=mybir.ActivationFunctionType.Exp,
                                 bias=negmax, scale=1.0, accum_out=rowsum)
            denom = smalls.tile([P, 1], FP32, tag="denom")
            nc.gpsimd.partition_all_reduce(denom, rowsum, channels=P,
                                           reduce_op=bass_isa.ReduceOp.add)
            inv = smalls.tile([P, 1], FP32, tag="inv")
            nc.vector.reciprocal(inv, denom)
            w_t = wp.tile([P, N_TILES], FP32, tag="w")
            nc.vector.tensor_scalar_mul(out=w_t, in0=exp_t, scalar1=inv)

            # weight output: 32x32 block transpose, then contiguous DMA
            wblk = wblkp.tile([P, N_TILES], FP32, tag="wblk")
            nc.vector.transpose(wblk, w_t)
            # wblk[32a+i, j] = weight[i*128 + 32a + j]
            wt_view = wt_out[b].rearrange("(i a j) -> a i j", i=32, a=4, j=32)
            nc.sync.dma_start(out=wt_view, in_=wblk)

            # context: PSUM-accumulated matmuls over the resident keys tiles
            ctx_lo = psum_ctx.tile([1, 512], FP32, tag="ctx")
            ctx_hi = psum_ctx.tile([1, 512], FP32, tag="ctx")
            for t in range(N_TILES):
                nc.tensor.matmul(ctx_lo, lhsT=w_t[:, t : t + 1],
                                 rhs=ktiles[t][:, 0:512],
                                 start=(t == 0), stop=(t == N_TILES - 1))
                nc.tensor.matmul(ctx_hi, lhsT=w_t[:, t : t + 1],
                                 rhs=ktiles[t][:, 512:1024],
                                 start=(t == 0), stop=(t == N_TILES - 1))
            ctx_sb = ctxsbp.tile([1, H], FP32, tag="ctxsb")
            nc.scalar.copy(ctx_sb[:, 0:512], ctx_lo)
            nc.scalar.copy(ctx_sb[:, 512:1024], ctx_hi)
            nc.sync.dma_start(out=ctx_out[b : b + 1, :], in_=ctx_sb)


_NC_CACHE = {}


def build_program(repeat: int = 1):
    if repeat in _NC_CACHE:
        return _NC_CACHE[repeat]
    nc = bacc.Bacc("TRN2", target_bir_lowering=False, debug=False,
                   enable_asserts=False, name=f"attn_r{repeat}")
    with tile.TileContext(nc) as tc:
        _emit(tc, repeat=repeat)
    nc.compile()
    _NC_CACHE[repeat] = nc
    return nc


def make_in_maps(inputs):
    keys = np.ascontiguousarray(np.asarray(inputs["keys"], dtype=np.float32))
    U = np.ascontiguousarray(np.asarray(inputs["U"], dtype=np.float32))
    V = np.ascontiguousarray(np.asarray(inputs["V"], dtype=np.float32))
    in_maps = []
    for c in range(N_CORES):
        in_maps.append({
            "keys": np.ascontiguousarray(keys[c * B_LOC : (c + 1) * B_LOC]),
            "U": U,
            "V": V,
        })
    return in_maps


def kernel(**inputs):
    # query / W provably do not affect the output (softmax shift invariance).
    nc = build_program()
    in_maps = make_in_maps(inputs)
    res = run_bass_kernel_spmd(nc, in_maps, core_ids=list(range(N_CORES)))
    ctx = np.concatenate([r["ctx_out"] for r in res.results], axis=0)
    wt = np.concatenate([r["wt_out"] for r in res.results], axis=0)
    context = ctx.reshape(B, 1, H).astype(np.float32)
    weight = wt.reshape(B, 1, S).astype(np.float32)
    return (context, weight)
